# revision 2
# baseline (speedup 1.0000x reference)
"""Causal single-head attention (B=8, T=2048, D=1024, HS=64) on 8 TRN2 NeuronCores.

Sharding: data-parallel over batch -- core b computes batch b end-to-end.
No collectives; outputs are concatenated on the host.

v2: restructured for PE density. Key changes vs v1:
  - supers processed in PAIRS (1024 q-cols) so consecutive matmuls share one
    stationary operand: walrus dedups the LDWEIGHTS (measured ~107ns each,
    fully serialized with matmuls on HW) -> proj 64->16 LDW, S/PV 80->48.
  - projections chunk-major inside a pair: for each D-chunk, [Wq|Wk] then
    [Wv|0] stationary, 2 matmuls each (super a, super b) into 2-bank PSUM
    tiles; accumulation groups interleave (skip_group_check).
  - no fp32 warmup (v1 burned ~6.8us of PE queue at cold clock); 2 dep-free
    bf16 matmuls kick the HAM activity window during the initial DMA.
  - phase order A01 B01 A23 B23: pair01's exp chain (ScalarE) overlaps
    pair23's projections (PE), since exp is the co-bottleneck (~21us vs
    ~40us PE).
  - epilogue in bf16 (transposes at 1 cyc/row), V' PSUM->SBUF copies on
    ScalarE (idle during proj), masks stay on GPSIMD.

Per-pair attention, k-tiles 0..nk-1 (nk = 8 for pair01, 16 for pair23):
    S^T[k, 0:512]    = K-tile_j @ Q^T-super_a   (only while j < 4a+4)
    S^T[k, 512:1024] = K-tile_j @ Q^T-super_b   (shared LDW)
    P^T = exp(S^T)            ScalarE, bf16 out; logits ~N(0,1), no max sub
    causal mask on diagonal tiles (GPSIMD affine_select, per 512 half)
    ot[65, half] += V'_j^T @ P^T-half           (row 64 of V' = ones = denom)
  epilogue: DVE copy ot->bf16, PE-transpose back, reciprocal + scale, DMA.
"""

import sys

if "/opt/trn_rl_repo" not in sys.path:
    sys.path.insert(0, "/opt/trn_rl_repo")

import os
from contextlib import ExitStack

import numpy as np

import concourse.bass as bass
import concourse.tile as tile
from concourse import bacc, mybir
from concourse.bass_utils import run_bass_kernel_spmd

B, T, D, HS = 8, 2048, 1024, 64
N_CORES = 8
F32 = mybir.dt.float32
BF16 = mybir.dt.bfloat16

TT = 128            # t/k tile (partition dim)
NDT = D // TT       # 8 contraction chunks
NTT = T // TT       # 16 k-tiles
QS = 512            # per-super matmul free dim (PSUM bank limit)
PW = 2 * QS         # pair width (2 supers)
NP = T // PW        # 2 super-pairs
VP = HS + 1         # V' width (64 + ones column)


def build_graph() -> bacc.Bacc:
    nc = bacc.Bacc("TRN2", target_bir_lowering=False, debug=False)

    xt_ext = nc.dram_tensor("xt", [D, T], BF16, kind="ExternalInput").ap()
    # wqkv[d, 0:128] = [Wq/8 | Wk]; wqkv[d, 128:256] = [Wv | 0]   (bf16)
    w_ext = nc.dram_tensor("wqkv", [D, 2 * TT], BF16, kind="ExternalInput").ap()
    # fp32 bias columns: col0[0:64]=bq/8, col0[64:128]=bk, col1[0:64]=bv,
    # col1[64]=1.0 (ones row for V' via the W zero-pad column)
    bcol_ext = nc.dram_tensor("bcol", [TT, 2], F32, kind="ExternalInput").ap()
    idb_ext = nc.dram_tensor("identb", [TT, TT], BF16, kind="ExternalInput").ap()
    out_ext = nc.dram_tensor("out", [T, HS], F32, kind="ExternalOutput").ap()

    with tile.TileContext(nc) as tc, ExitStack() as ctx:
        const = ctx.enter_context(tc.tile_pool(name="const", bufs=1))
        persist = ctx.enter_context(tc.tile_pool(name="persist", bufs=1))
        xt_pool = ctx.enter_context(tc.tile_pool(name="xt", bufs=1))
        vt_pool = ctx.enter_context(tc.tile_pool(name="vt", bufs=2))
        pt_pool = ctx.enter_context(tc.tile_pool(name="pt", bufs=3))
        otsb_pool = ctx.enter_context(tc.tile_pool(name="otsb", bufs=2))
        osb_pool = ctx.enter_context(tc.tile_pool(name="osb", bufs=2))
        rc_pool = ctx.enter_context(tc.tile_pool(name="rc", bufs=2))
        warm_pool = ctx.enter_context(tc.tile_pool(name="warm", bufs=1))
        psum = ctx.enter_context(tc.tile_pool(name="ps", bufs=1, space="PSUM"))

        # PSUM budget (8 banks): tag "b2" bufs=3 x [128,1024]f32 (2 banks) = 6
        # + tag "acc" bufs=1 x [65,1024]f32 = 2.  Small bf16 transpose tiles
        # also draw from "b2" (slot sized by the largest user).
        def b2(name, shape=None, dtype=F32):
            return psum.tile(
                shape or [TT, PW], dtype, tag="b2", bufs=3, name=name
            )

        # ---- constants ----
        idb_sb = const.tile([TT, TT], BF16)
        nc.sync.dma_start(idb_sb[:], idb_ext)
        bcol_sb = const.tile([TT, 2], F32)
        nc.sync.dma_start(bcol_sb[:], bcol_ext)
        w_sb = const.tile([TT, NDT * 2 * TT], BF16)
        nc.sync.dma_start(
            w_sb[:].rearrange("p (c n) -> p c n", c=NDT),
            w_ext.rearrange("(c p) n -> p c n", p=TT),
        )

        # ---- persistent per-core intermediates (bf16 matmul operands) ----
        qt_sb = persist.tile([HS, T], BF16)         # Q^T / 8 (scale folded)
        kt_sb = persist.tile([HS, T], BF16)         # K^T
        vp_sb = persist.tile([TT, NTT * VP], BF16)  # V' [128, 65] per k-tile

        # ---- x^T chunk-halves, all resident (32KB/partition) ----
        xt_sb = [
            [
                xt_pool.tile([TT, PW], BF16, tag=f"xt{c}_{p}", name=f"xt{c}_{p}")
                for p in range(NP)
            ]
            for c in range(NDT)
        ]
        for p in range(NP):
            for c in range(NDT):
                nc.sync.dma_start(
                    xt_sb[c][p][:], xt_ext[c * TT:(c + 1) * TT,
                                           p * PW:(p + 1) * PW]
                )

        # ---- HAM kick: 2 dep-free bf16 matmuls during the initial DMA ----
        warm_sb = warm_pool.tile([TT, QS], BF16)
        nc.gpsimd.memset(warm_sb[:], 0.0)
        warm_ps = b2("warm_ps")
        for _ in range(2):
            nc.tensor.matmul(
                warm_ps[:, 0:QS], warm_sb[:, 0:TT], warm_sb[:],
                start=True, stop=True,
            )

        def do_proj(p):
            """Projections for super-pair p: chunk-major, shared stationary."""
            a_sl = slice(2 * p * QS, (2 * p + 1) * QS)
            b_sl = slice((2 * p + 1) * QS, (2 * p + 2) * QS)
            pqk = b2(f"pqk{p}")
            pv = b2(f"pv{p}")
            for c in range(NDT):
                xa = xt_sb[c][p][:, 0:QS]
                xb = xt_sb[c][p][:, QS:PW]
                for half, pp in ((0, pqk), (1, pv)):
                    wsl = w_sb[:, c * 2 * TT + half * TT:c * 2 * TT + (half + 1) * TT]
                    for xi, x in enumerate((xa, xb)):
                        nc.tensor.matmul(
                            pp[:, xi * QS:(xi + 1) * QS], wsl, x,
                            start=(c == 0), stop=(c == NDT - 1),
                            skip_group_check=True,
                        )
            # rows 0:64 = Q^T/8, rows 64:128 = K^T (per-partition bias)
            nc.vector.tensor_scalar_add(
                qt_sb[:, 2 * p * QS:(2 * p + 2) * QS], pqk[0:HS, :],
                bcol_sb[0:HS, 0:1],
            )
            nc.vector.tensor_scalar_add(
                kt_sb[:, 2 * p * QS:(2 * p + 2) * QS], pqk[HS:2 * HS, :],
                bcol_sb[HS:2 * HS, 0:1],
            )
            # rows 0:64 = V^T + bv; row 64 = 0 (W zero-pad) + 1.0
            vt = vt_pool.tile([VP, PW], BF16, tag="vt", name=f"vt{p}")
            nc.vector.tensor_scalar_add(vt[0:VP, :], pv[0:VP, :], bcol_sb[0:VP, 1:2])
            for u in range(8):
                j = 8 * p + u
                vps = b2(f"vps{j}", [TT, VP], BF16)
                nc.tensor.transpose(
                    vps[:], vt[:, u * TT:(u + 1) * TT], idb_sb[0:VP, 0:VP]
                )
                nc.scalar.copy(vp_sb[:, j * VP:(j + 1) * VP], vps[:])

        def do_attn(p):
            """Causal attention for super-pair p (supers a=2p, b=2p+1)."""
            a, b = 2 * p, 2 * p + 1
            nk = 4 * b + 4          # k-tiles visible to super b
            nka = 4 * a + 4         # k-tiles visible to super a
            ot = psum.tile([VP, PW], F32, tag="acc", bufs=1, name=f"ot{p}")
            ptiles = {}

            def emit_s(j):
                wide = j < nka
                sp = b2(f"sp{p}_{j}")
                ksl = kt_sb[:, j * TT:(j + 1) * TT]
                if wide:
                    nc.tensor.matmul(
                        sp[:, 0:QS], ksl, qt_sb[:, a * QS:(a + 1) * QS],
                        start=True, stop=True,
                    )
                nc.tensor.matmul(
                    sp[:, QS:PW], ksl, qt_sb[:, b * QS:(b + 1) * QS],
                    start=True, stop=True,
                )
                pt = pt_pool.tile([TT, PW], BF16, tag="pt", name=f"pt{p}_{j}")
                if wide:
                    nc.scalar.activation(
                        pt[:], sp[:], mybir.ActivationFunctionType.Exp
                    )
                else:
                    nc.scalar.activation(
                        pt[:, QS:PW], sp[:, QS:PW],
                        mybir.ActivationFunctionType.Exp,
                    )
                # diagonal tiles: zero P^T[kk, qq] where qq < kk + 128*dd
                for half, s in ((0, a), (1, b)):
                    dd = j - 4 * s
                    if 0 <= dd < 4:
                        nc.gpsimd.affine_select(
                            out=pt[:, half * QS:(half + 1) * QS],
                            in_=pt[:, half * QS:(half + 1) * QS],
                            compare_op=mybir.AluOpType.is_ge,
                            fill=0.0,
                            base=-TT * dd,
                            channel_multiplier=-1,
                            pattern=[[1, QS]],
                        )
                ptiles[j] = pt

            def emit_pv(j):
                pt = ptiles.pop(j)
                vsl = vp_sb[:, j * VP:(j + 1) * VP]
                if j < nka:
                    nc.tensor.matmul(
                        ot[:, 0:QS], vsl, pt[:, 0:QS],
                        start=(j == 0), stop=(j == nka - 1),
                        skip_group_check=True,
                    )
                nc.tensor.matmul(
                    ot[:, QS:PW], vsl, pt[:, QS:PW],
                    start=(j == 0), stop=(j == nk - 1),
                    skip_group_check=True,
                )

            emit_s(0)
            emit_s(1)
            for j in range(nk):
                if j + 2 < nk:
                    emit_s(j + 2)
                emit_pv(j)

            # -- epilogue (bf16): normalize + transpose back + store --
            ot_sb = otsb_pool.tile([VP, PW], BF16, tag="otsb", name=f"otsb{p}")
            nc.vector.tensor_copy(ot_sb[:], ot[:])
            o_sb = osb_pool.tile([TT, 8 * HS], F32, tag="osb", name=f"osb{p}")
            for u in range(8):
                ob = b2(f"ob{p}_{u}", [TT, VP], BF16)
                nc.tensor.transpose(
                    ob[:], ot_sb[:, u * TT:(u + 1) * TT], idb_sb[0:VP, 0:VP]
                )
                rc = rc_pool.tile([TT, 1], F32, tag="rc", name=f"rc{p}_{u}")
                nc.vector.reciprocal(rc[:], ob[:, HS:HS + 1])
                nc.vector.tensor_scalar_mul(
                    o_sb[:, u * HS:(u + 1) * HS], ob[:, 0:HS], rc[:]
                )
            nc.sync.dma_start(
                out_ext[p * PW:(p + 1) * PW, :].rearrange("(u p) h -> p u h", p=TT),
                o_sb[:].rearrange("p (u h) -> p u h", u=8),
            )

        do_proj(0)
        do_attn(0)
        do_proj(1)
        do_attn(1)

    nc.compile()
    return nc


def make_inputs(x_b, Wq, bq, Wk, bk, Wv, bv):
    """Host-side prep for one core's in_map (x_b: [T, D] fp32)."""
    import ml_dtypes

    bf = ml_dtypes.bfloat16
    scale = 1.0 / np.sqrt(np.float32(HS))
    w = np.zeros((D, 2 * TT), dtype=np.float32)
    w[:, 0:HS] = Wq * scale
    w[:, HS:2 * HS] = Wk
    w[:, 2 * HS:3 * HS] = Wv
    bcol = np.zeros((TT, 2), dtype=np.float32)
    bcol[0:HS, 0] = bq * scale
    bcol[HS:2 * HS, 0] = bk
    bcol[0:HS, 1] = bv
    bcol[HS, 1] = 1.0
    identb = np.eye(TT, dtype=bf)
    return {
        "xt": np.ascontiguousarray(x_b.T).astype(bf),
        "wqkv": w.astype(bf),
        "bcol": bcol,
        "identb": identb,
    }


_NC_CACHE = None


def _get_nc():
    global _NC_CACHE
    if _NC_CACHE is None:
        _NC_CACHE = build_graph()
    return _NC_CACHE


def kernel(x, Wq, bq, Wk, bk, Wv, bv):
    x = np.asarray(x, dtype=np.float32)
    args = [np.asarray(a, dtype=np.float32) for a in (Wq, bq, Wk, bk, Wv, bv)]
    nc = _get_nc()
    in_maps = [make_inputs(x[b], *args) for b in range(N_CORES)]
    trace = os.environ.get("BASS_ATTN_TRACE", "0") == "1"
    res = run_bass_kernel_spmd(
        nc, in_maps, core_ids=list(range(N_CORES)), trace=trace
    )
    if trace:
        print(
            f"HW exec time: {res.exec_time_ns} ns "
            f"(mean {res.mean_exec_time_ns}, max core {res.max_exec_time_core_id})"
        )
    out = np.stack([res.results[b]["out"] for b in range(N_CORES)], axis=0)
    return out


# revision 6
# speedup vs baseline: 1.0298x; 1.0298x over previous
"""Causal single-head attention (B=8, T=2048, D=1024, HS=64) on 8 TRN2 NeuronCores.

Sharding: data-parallel over batch -- core b computes batch b end-to-end.
No collectives; outputs are post-processed (normalize + transpose) on host.

v3 notes (learned from v1/v2 traces):
  - ~6.6us fixed framework preamble before any user instruction; exec_time
    includes it.  DMA issues serialize ~650ns each on the issuing engine's
    queue, and queues process transfers in order -> w must be a contiguous
    DMA (host pre-arranges layout) and x chunks split across the two HWDGE
    queues (sync + scalar) so the first projection starts ~8us.
  - HAM clock gate: grant of K=8/8 needs a ~100%-busy 3.4us window; dropping
    below ~50% busy re-throttles to K=4/8 and re-grant is unreliable.  So:
    dep-free bf16 warmup covers preamble->first-chunk, phases are
    interleaved (pair23 QK projections woven into pair01 attention) so the
    PE never idles while ScalarE exp chains run, and the device-side
    epilogue is deleted entirely.
  - LDWEIGHTS is emitted per matmul (no dedup) but hides under the moving
    stream when the PE queue stays full at K=8.
  - Output: device stores numerator^T and denominator rows [65, T] fp32;
    host divides + transposes (better precision than on-chip bf16 epilogue,
    and saves 16 PE transposes + DVE chain + 5us of tail).

Per-pair attention (supers a=2p, b=2p+1), k-tiles j < nk = 8|16:
    S^T[k, 0:512]    = K_j @ Q^T_a     (only while j < nka = 4a+4)
    S^T[k, 512:1024] = K_j @ Q^T_b
    P^T = exp(S^T)                     (ScalarE, bf16; logits ~N(0,1))
    causal mask on diagonal tiles      (GPSIMD affine_select, per half)
    ot[65, half] += V'_j^T @ P^T-half  (V' row 64 = ones -> denominator)
  each ot half: DVE copy -> SBUF fp32 -> DMA out as soon as it stops.
"""

import sys

if "/opt/trn_rl_repo" not in sys.path:
    sys.path.insert(0, "/opt/trn_rl_repo")

import os
from contextlib import ExitStack

import numpy as np

import concourse.bass as bass
import concourse.tile as tile
from concourse import bacc, mybir
from concourse.bass_utils import run_bass_kernel_spmd

B, T, D, HS = 8, 2048, 1024, 64
N_CORES = 8
F32 = mybir.dt.float32
BF16 = mybir.dt.bfloat16

TT = 128            # t/k tile (partition dim)
NDT = D // TT       # 8 contraction chunks
NTT = T // TT       # 16 k-tiles
QS = 512            # per-super matmul free dim (PSUM bank limit)
PW = 2 * QS         # pair width (2 supers)
NP = T // PW        # 2 super-pairs
VP = HS + 1         # V' width (64 + ones column)


def build_graph() -> bacc.Bacc:
    nc = bacc.Bacc("TRN2", target_bir_lowering=False, debug=False)

    xt_ext = nc.dram_tensor("xt", [D, T], BF16, kind="ExternalInput").ap()
    # host pre-arranged: w[p, c*256 + n] = wqkv[c*128+p, n]; per chunk c the
    # 256 cols are [Wq/8 | Wk | Wv | 0]  (contiguous DMA, 4KB lines)
    w_ext = nc.dram_tensor("wqkv", [TT, NDT * 2 * TT], BF16,
                           kind="ExternalInput").ap()
    bcol_ext = nc.dram_tensor("bcol", [TT, 2], F32, kind="ExternalInput").ap()
    idb_ext = nc.dram_tensor("identb", [TT, TT], BF16, kind="ExternalInput").ap()
    # rows 0:64 = (attn @ V)^T numerator, row 64 = softmax denominator
    out_ext = nc.dram_tensor("outT", [VP, T], F32, kind="ExternalOutput").ap()

    with tile.TileContext(nc) as tc, ExitStack() as ctx:
        const = ctx.enter_context(tc.tile_pool(name="const", bufs=1))
        persist = ctx.enter_context(tc.tile_pool(name="persist", bufs=1))
        xt_pool = ctx.enter_context(tc.tile_pool(name="xt", bufs=1))
        vt_pool = ctx.enter_context(tc.tile_pool(name="vt", bufs=2))
        pt_pool = ctx.enter_context(tc.tile_pool(name="pt", bufs=4))
        osb_pool = ctx.enter_context(tc.tile_pool(name="osb", bufs=2))
        warm_pool = ctx.enter_context(tc.tile_pool(name="warm", bufs=1))
        psum = ctx.enter_context(tc.tile_pool(name="ps", bufs=1, space="PSUM"))

        # PSUM (8 banks): tag "proj" bufs=2 x [128,1024]f32 (2 banks) holds
        # the live {pqk | pv | ot} pair; tag "sp" bufs=2 x 2 banks rotates
        # S^T tiles / V'-transpose temps / warmup.
        def proj_t(name):
            return psum.tile([TT, PW], F32, tag="proj", bufs=2, name=name)

        def sp_t(name, shape=None, dtype=F32):
            return psum.tile(shape or [TT, PW], dtype, tag="sp", bufs=2,
                             name=name)

        # ---- constants; small ones on the scalar queue ahead of x ----
        w_sb = const.tile([TT, NDT * 2 * TT], BF16)
        nc.sync.dma_start(w_sb[:], w_ext)
        bcol_sb = const.tile([TT, 2], F32)
        nc.scalar.dma_start(bcol_sb[:], bcol_ext)
        idb_sb = const.tile([TT, TT], BF16)
        nc.scalar.dma_start(idb_sb[:], idb_ext)

        # ---- persistent per-core intermediates (bf16 matmul operands) ----
        qt_sb = persist.tile([HS, T], BF16)         # Q^T / 8 (scale folded)
        kt_sb = persist.tile([HS, T], BF16)         # K^T
        vp_sb = persist.tile([TT, NTT * VP], BF16)  # V' [128, 65] per k-tile

        # ---- x^T chunk-halves: pair0 on the scalar queue (starts right
        # away), pair1 on the sync queue (behind w) ----
        xt_sb = [
            [
                xt_pool.tile([TT, PW], BF16, tag=f"xt{c}_{p}", name=f"xt{c}_{p}")
                for p in range(NP)
            ]
            for c in range(NDT)
        ]
        for c in range(NDT):
            nc.scalar.dma_start(
                xt_sb[c][0][:], xt_ext[c * TT:(c + 1) * TT, 0:PW]
            )
        for c in range(NDT):
            nc.sync.dma_start(
                xt_sb[c][1][:], xt_ext[c * TT:(c + 1) * TT, PW:2 * PW]
            )

        # ---- HAM kick: dep-free bf16 matmuls fill preamble->first chunk ----
        warm_sb = warm_pool.tile([TT, QS], BF16)
        nc.gpsimd.memset(warm_sb[:], 0.0)
        warm_ps = sp_t("warm_ps", [TT, QS])
        for _ in range(4):
            nc.tensor.matmul(
                warm_ps[:], warm_sb[:, 0:TT], warm_sb[:],
                start=True, stop=True,
            )

        def proj_qk(p, pqk):
            """QK projection matmuls for pair p, one chunk-group per next()."""
            for c in range(NDT):
                wsl = w_sb[:, c * 2 * TT:c * 2 * TT + TT]
                for xi in range(2):
                    nc.tensor.matmul(
                        pqk[:, xi * QS:(xi + 1) * QS], wsl,
                        xt_sb[c][p][:, xi * QS:(xi + 1) * QS],
                        start=(c == 0), stop=(c == NDT - 1),
                        skip_group_check=True,
                    )
                yield

        def drain_qk(p, pqk):
            nc.vector.tensor_scalar_add(
                qt_sb[:, p * PW:(p + 1) * PW], pqk[0:HS, :], bcol_sb[0:HS, 0:1]
            )
            nc.vector.tensor_scalar_add(
                kt_sb[:, p * PW:(p + 1) * PW], pqk[HS:2 * HS, :],
                bcol_sb[HS:2 * HS, 0:1],
            )

        def proj_v(p):
            """V projection + V' transposes for pair p."""
            pv = proj_t(f"pv{p}")
            for c in range(NDT):
                wsl = w_sb[:, c * 2 * TT + TT:(c + 1) * 2 * TT]
                for xi in range(2):
                    nc.tensor.matmul(
                        pv[:, xi * QS:(xi + 1) * QS], wsl,
                        xt_sb[c][p][:, xi * QS:(xi + 1) * QS],
                        start=(c == 0), stop=(c == NDT - 1),
                        skip_group_check=True,
                    )
            vt = vt_pool.tile([VP, PW], BF16, tag="vt", name=f"vt{p}")
            nc.vector.tensor_scalar_add(vt[0:VP, :], pv[0:VP, :], bcol_sb[0:VP, 1:2])
            for u in range(8):
                j = 8 * p + u
                vps = sp_t(f"vps{j}", [TT, VP], BF16)
                nc.tensor.transpose(
                    vps[:], vt[:, u * TT:(u + 1) * TT], idb_sb[0:VP, 0:VP]
                )
                nc.scalar.copy(vp_sb[:, j * VP:(j + 1) * VP], vps[:])

        def attn(p, filler=None):
            """Attention for pair p; `filler` generator yields PE work to
            interleave (one item per k-tile) so exp chains never starve it."""
            a, b = 2 * p, 2 * p + 1
            nka, nk = 4 * a + 4, 4 * b + 4
            ot = psum.tile([VP, PW], F32, tag="proj", bufs=2, name=f"ot{p}")
            ptiles = {}

            def emit_s(j):
                wide = j < nka
                sp = sp_t(f"sp{p}_{j}")
                ksl = kt_sb[:, j * TT:(j + 1) * TT]
                if wide:
                    nc.tensor.matmul(
                        sp[:, 0:QS], ksl, qt_sb[:, a * QS:(a + 1) * QS],
                        start=True, stop=True,
                    )
                nc.tensor.matmul(
                    sp[:, QS:PW], ksl, qt_sb[:, b * QS:(b + 1) * QS],
                    start=True, stop=True,
                )
                pt = pt_pool.tile([TT, PW], BF16, tag="pt", name=f"pt{p}_{j}")
                if wide:
                    nc.scalar.activation(
                        pt[:], sp[:], mybir.ActivationFunctionType.Exp
                    )
                else:
                    nc.scalar.activation(
                        pt[:, QS:PW], sp[:, QS:PW],
                        mybir.ActivationFunctionType.Exp,
                    )
                for half, s in ((0, a), (1, b)):
                    dd = j - 4 * s
                    if 0 <= dd < 4:
                        nc.gpsimd.affine_select(
                            out=pt[:, half * QS:(half + 1) * QS],
                            in_=pt[:, half * QS:(half + 1) * QS],
                            compare_op=mybir.AluOpType.is_ge,
                            fill=0.0,
                            base=-TT * dd,
                            channel_multiplier=-1,
                            pattern=[[1, QS]],
                        )
                ptiles[j] = pt

            def emit_pv(j):
                pt = ptiles.pop(j)
                vsl = vp_sb[:, j * VP:(j + 1) * VP]
                if j < nka:
                    nc.tensor.matmul(
                        ot[:, 0:QS], vsl, pt[:, 0:QS],
                        start=(j == 0), stop=(j == nka - 1),
                        skip_group_check=True,
                    )
                nc.tensor.matmul(
                    ot[:, QS:PW], vsl, pt[:, QS:PW],
                    start=(j == 0), stop=(j == nk - 1),
                    skip_group_check=True,
                )

            def store_half(half):
                osb = osb_pool.tile([VP, QS], F32, tag="osb",
                                    name=f"osb{p}_{half}")
                nc.vector.tensor_copy(osb[:], ot[:, half * QS:(half + 1) * QS])
                nc.sync.dma_start(
                    out_ext[:, (2 * p + half) * QS:(2 * p + half + 1) * QS],
                    osb[:],
                )

            emit_s(0)
            emit_s(1)
            for j in range(nk):
                if filler is not None:
                    next(filler, None)
                if j + 2 < nk:
                    emit_s(j + 2)
                emit_pv(j)
                if j == nka - 1:
                    store_half(0)
            store_half(1)

        # ---- schedule ----
        pqk0 = proj_t("pqk0")
        for _ in proj_qk(0, pqk0):
            pass
        drain_qk(0, pqk0)
        proj_v(0)

        pqk1 = proj_t("pqk1")
        attn(0, filler=proj_qk(1, pqk1))
        drain_qk(1, pqk1)
        proj_v(1)
        attn(1)

    nc.compile()
    return nc


def make_inputs(x_b, Wq, bq, Wk, bk, Wv, bv):
    """Host-side prep for one core's in_map (x_b: [T, D] fp32)."""
    import ml_dtypes

    bf = ml_dtypes.bfloat16
    scale = 1.0 / np.sqrt(np.float32(HS))
    w = np.zeros((D, 2 * TT), dtype=np.float32)
    w[:, 0:HS] = Wq * scale
    w[:, HS:2 * HS] = Wk
    w[:, 2 * HS:3 * HS] = Wv
    # pre-arrange for a contiguous [128, 2048] DMA: w2[p, c, :] = w[c*128+p, :]
    w2 = np.ascontiguousarray(
        w.reshape(NDT, TT, 2 * TT).transpose(1, 0, 2).reshape(TT, NDT * 2 * TT)
    )
    bcol = np.zeros((TT, 2), dtype=np.float32)
    bcol[0:HS, 0] = bq * scale
    bcol[HS:2 * HS, 0] = bk
    bcol[0:HS, 1] = bv
    bcol[HS, 1] = 1.0
    identb = np.eye(TT, dtype=bf)
    return {
        "xt": np.ascontiguousarray(x_b.T).astype(bf),
        "wqkv": w2.astype(bf),
        "bcol": bcol,
        "identb": identb,
    }


def finish_output(outT):
    """Host-side normalize + transpose: outT [65, T] -> [T, HS]."""
    o = np.asarray(outT, dtype=np.float32)
    return (o[0:HS, :] / o[HS:HS + 1, :]).T


_NC_CACHE = None


def _get_nc():
    global _NC_CACHE
    if _NC_CACHE is None:
        _NC_CACHE = build_graph()
    return _NC_CACHE


def kernel(x, Wq, bq, Wk, bk, Wv, bv):
    x = np.asarray(x, dtype=np.float32)
    args = [np.asarray(a, dtype=np.float32) for a in (Wq, bq, Wk, bk, Wv, bv)]
    nc = _get_nc()
    in_maps = [make_inputs(x[b], *args) for b in range(N_CORES)]
    trace = os.environ.get("BASS_ATTN_TRACE", "0") == "1"
    res = run_bass_kernel_spmd(
        nc, in_maps, core_ids=list(range(N_CORES)), trace=trace
    )
    if trace:
        print(
            f"HW exec time: {res.exec_time_ns} ns "
            f"(mean {res.mean_exec_time_ns}, max core {res.max_exec_time_core_id})"
        )
    out = np.stack(
        [finish_output(res.results[b]["outT"]) for b in range(N_CORES)], axis=0
    )
    return out


# revision 7
# speedup vs baseline: 1.2342x; 1.1985x over previous
"""Causal single-head attention (B=8, T=2048, D=1024, HS=64) on 8 TRN2 NeuronCores.

Sharding: data-parallel over batch -- core b computes batch b end-to-end.
No collectives; outputs are post-processed (normalize + transpose) on host.

v4 notes (hard-won from v1-v3 traces):
  - ~6.6us fixed framework preamble; DMA issues ~0.7-1.3us each on the
    issuing queue, transfers contend chip-wide (8 cores share HBM/DMA:
    ~1.3us per 256KB chunk in practice, not the 0.7us single-core number).
  - HAM clock gate: K=8/8 grant needs a ~100%-busy 3.4us window; any later
    window under ~60% busy re-throttles to K=4/8 and re-grant is unreliable
    (v2/v3 ran entire phases 100%-busy at K=4, stuck).  Strategy: one dense
    stream, no phase-boundary bubbles.
  - DMA order: x chunk0 leads the scalar queue, W(QK half) leads the sync
    queue, so the first projection starts ~9us; 4 dep-free bf16 warmup
    matmuls bridge preamble -> first chunk and earn the grant early.
  - Boundary bridges: S^T matmuls of the next attention pair are emitted
    between the V-projection matmuls and the V' transposes (covering the
    DVE vt-add latency); pair1 QK projections are interleaved into pair0's
    attention as PE filler; pair0's final store is deferred behind
    drain_qk(1) so kt pair1 is ready sooner.
  - No device epilogue: numerator^T [64,T] + denominator [1,T] go to DRAM
    in fp32; the host divides + transposes (also better precision).

Per-pair attention (supers a=2p, b=2p+1), k-tiles j < nk = 8|16:
    S^T[k, 0:512]    = K_j @ Q^T_a     (only while j < nka = 4a+4)
    S^T[k, 512:1024] = K_j @ Q^T_b
    P^T = exp(S^T)                     (ScalarE, bf16; logits ~N(0,1))
    causal mask on diagonal tiles      (GPSIMD affine_select, per half)
    ot[65, half] += V'_j^T @ P^T-half  (V' row 64 = ones -> denominator)
"""

import sys

if "/opt/trn_rl_repo" not in sys.path:
    sys.path.insert(0, "/opt/trn_rl_repo")

import os
from contextlib import ExitStack

import numpy as np

import concourse.bass as bass
import concourse.tile as tile
from concourse import bacc, mybir
from concourse.bass_utils import run_bass_kernel_spmd

B, T, D, HS = 8, 2048, 1024, 64
N_CORES = 8
F32 = mybir.dt.float32
BF16 = mybir.dt.bfloat16

TT = 128            # t/k tile (partition dim)
NDT = D // TT       # 8 contraction chunks
NTT = T // TT       # 16 k-tiles
QS = 512            # per-super matmul free dim (PSUM bank limit)
PW = 2 * QS         # pair width (2 supers)
NP = T // PW        # 2 super-pairs
VP = HS + 1         # V' width (64 + ones column)


def build_graph() -> bacc.Bacc:
    nc = bacc.Bacc("TRN2", target_bir_lowering=False, debug=False)

    xt_ext = nc.dram_tensor("xt", [D, T], BF16, kind="ExternalInput").ap()
    # host pre-arranged, split by half for early availability:
    # wqk[p, c*128 + n] = [Wq/8 | Wk][c*128+p, n];  wv likewise [Wv | 0]
    wqk_ext = nc.dram_tensor("wqk", [TT, NDT * TT], BF16,
                             kind="ExternalInput").ap()
    wv_ext = nc.dram_tensor("wv", [TT, NDT * TT], BF16,
                            kind="ExternalInput").ap()
    bcol_ext = nc.dram_tensor("bcol", [TT, 2], F32, kind="ExternalInput").ap()
    idb_ext = nc.dram_tensor("identb", [TT, TT], BF16, kind="ExternalInput").ap()
    # rows 0:64 = (attn @ V)^T numerator, row 64 = softmax denominator
    out_ext = nc.dram_tensor("outT", [VP, T], F32, kind="ExternalOutput").ap()

    with tile.TileContext(nc) as tc, ExitStack() as ctx:
        const = ctx.enter_context(tc.tile_pool(name="const", bufs=1))
        persist = ctx.enter_context(tc.tile_pool(name="persist", bufs=1))
        xt_pool = ctx.enter_context(tc.tile_pool(name="xt", bufs=1))
        vt_pool = ctx.enter_context(tc.tile_pool(name="vt", bufs=2))
        pt_pool = ctx.enter_context(tc.tile_pool(name="pt", bufs=4))
        osb_pool = ctx.enter_context(tc.tile_pool(name="osb", bufs=2))
        warm_pool = ctx.enter_context(tc.tile_pool(name="warm", bufs=1))
        psum = ctx.enter_context(tc.tile_pool(name="ps", bufs=1, space="PSUM"))

        # PSUM (8 banks): tag "proj" bufs=2 x 2 banks holds the live
        # {pqk | pv | ot} set; tag "sp" bufs=2 x 2 banks rotates S^T tiles /
        # V'-transpose temps / warmup.
        def proj_t(name, shape=None):
            return psum.tile(shape or [TT, PW], F32, tag="proj", bufs=2,
                             name=name)

        def sp_t(name, shape=None, dtype=F32):
            return psum.tile(shape or [TT, PW], dtype, tag="sp", bufs=2,
                             name=name)

        # ---- persistent per-core intermediates (bf16 matmul operands) ----
        qt_sb = persist.tile([HS, T], BF16)         # Q^T / 8 (scale folded)
        kt_sb = persist.tile([HS, T], BF16)         # K^T
        vp_sb = persist.tile([TT, NTT * VP], BF16)  # V' [128, 65] per k-tile

        # ---- DMAs.  scalar queue: x pair0 chunks first (first MM input),
        # then the small consts.  sync queue: W halves first, then x pair1.
        xt_sb = [
            [
                xt_pool.tile([TT, PW], BF16, tag=f"xt{c}_{p}", name=f"xt{c}_{p}")
                for p in range(NP)
            ]
            for c in range(NDT)
        ]
        wqk_sb = const.tile([TT, NDT * TT], BF16)
        wv_sb = const.tile([TT, NDT * TT], BF16)
        bcol_sb = const.tile([TT, 2], F32)
        idb_sb = const.tile([TT, TT], BF16)

        for c in range(NDT):
            nc.scalar.dma_start(
                xt_sb[c][0][:], xt_ext[c * TT:(c + 1) * TT, 0:PW]
            )
        nc.scalar.dma_start(bcol_sb[:], bcol_ext)
        nc.scalar.dma_start(idb_sb[:], idb_ext)

        nc.sync.dma_start(wqk_sb[:], wqk_ext)
        nc.sync.dma_start(wv_sb[:], wv_ext)
        for c in range(NDT):
            nc.sync.dma_start(
                xt_sb[c][1][:], xt_ext[c * TT:(c + 1) * TT, PW:2 * PW]
            )

        # ---- HAM kick: dep-free bf16 matmuls bridge preamble -> chunk0 ----
        warm_sb = warm_pool.tile([TT, QS], BF16)
        nc.gpsimd.memset(warm_sb[:], 0.0)
        warm_ps = sp_t("warm_ps", [TT, QS])
        for _ in range(4):
            nc.tensor.matmul(
                warm_ps[:], warm_sb[:, 0:TT], warm_sb[:],
                start=True, stop=True,
            )

        def proj_half(p, w_sb, pp):
            """One half-projection (8 chunk-groups) for pair p; yields after
            each chunk so it can double as attention PE filler."""
            for c in range(NDT):
                wsl = w_sb[:, c * TT:(c + 1) * TT]
                for xi in range(2):
                    nc.tensor.matmul(
                        pp[:, xi * QS:(xi + 1) * QS], wsl,
                        xt_sb[c][p][:, xi * QS:(xi + 1) * QS],
                        start=(c == 0), stop=(c == NDT - 1),
                        skip_group_check=True,
                    )
                yield

        def drain_qk(p, pqk):
            nc.vector.tensor_scalar_add(
                qt_sb[:, p * PW:(p + 1) * PW], pqk[0:HS, :], bcol_sb[0:HS, 0:1]
            )
            nc.vector.tensor_scalar_add(
                kt_sb[:, p * PW:(p + 1) * PW], pqk[HS:2 * HS, :],
                bcol_sb[HS:2 * HS, 0:1],
            )

        def vp_finish(p, pv):
            """V bias add + V' PE-transposes + copies for pair p."""
            vt = vt_pool.tile([VP, PW], BF16, tag="vt", name=f"vt{p}")
            nc.vector.tensor_scalar_add(vt[0:VP, :], pv[0:VP, :], bcol_sb[0:VP, 1:2])
            for u in range(8):
                j = 8 * p + u
                vps = sp_t(f"vps{j}", [TT, VP], BF16)
                nc.tensor.transpose(
                    vps[:], vt[:, u * TT:(u + 1) * TT], idb_sb[0:VP, 0:VP]
                )
                nc.scalar.copy(vp_sb[:, j * VP:(j + 1) * VP], vps[:])

        def attn(p, pre=None, filler=None):
            """Attention for pair p.  `pre` emits this pair's V'-transpose
            block after S0/S1 (bridging the vt-add latency); `filler` yields
            one chunk of next-pair projection per k-tile as PE filler.
            Returns the deferred b-half store closure."""
            a, b = 2 * p, 2 * p + 1
            nka, nk = 4 * a + 4, 4 * b + 4
            ot = proj_t(f"ot{p}", [VP, PW])
            ptiles = {}

            def emit_s(j):
                wide = j < nka
                sp = sp_t(f"sp{p}_{j}")
                ksl = kt_sb[:, j * TT:(j + 1) * TT]
                if wide:
                    nc.tensor.matmul(
                        sp[:, 0:QS], ksl, qt_sb[:, a * QS:(a + 1) * QS],
                        start=True, stop=True,
                    )
                nc.tensor.matmul(
                    sp[:, QS:PW], ksl, qt_sb[:, b * QS:(b + 1) * QS],
                    start=True, stop=True,
                )
                pt = pt_pool.tile([TT, PW], BF16, tag="pt", name=f"pt{p}_{j}")
                if wide:
                    nc.scalar.activation(
                        pt[:], sp[:], mybir.ActivationFunctionType.Exp
                    )
                else:
                    nc.scalar.activation(
                        pt[:, QS:PW], sp[:, QS:PW],
                        mybir.ActivationFunctionType.Exp,
                    )
                for half, s in ((0, a), (1, b)):
                    dd = j - 4 * s
                    if 0 <= dd < 4:
                        nc.gpsimd.affine_select(
                            out=pt[:, half * QS:(half + 1) * QS],
                            in_=pt[:, half * QS:(half + 1) * QS],
                            compare_op=mybir.AluOpType.is_ge,
                            fill=0.0,
                            base=-TT * dd,
                            channel_multiplier=-1,
                            pattern=[[1, QS]],
                        )
                ptiles[j] = pt

            def emit_pv(j):
                pt = ptiles.pop(j)
                vsl = vp_sb[:, j * VP:(j + 1) * VP]
                if j < nka:
                    nc.tensor.matmul(
                        ot[:, 0:QS], vsl, pt[:, 0:QS],
                        start=(j == 0), stop=(j == nka - 1),
                        skip_group_check=True,
                    )
                nc.tensor.matmul(
                    ot[:, QS:PW], vsl, pt[:, QS:PW],
                    start=(j == 0), stop=(j == nk - 1),
                    skip_group_check=True,
                )

            def store_half(half):
                osb = osb_pool.tile([VP, QS], F32, tag="osb",
                                    name=f"osb{p}_{half}")
                nc.vector.tensor_copy(osb[:], ot[:, half * QS:(half + 1) * QS])
                nc.sync.dma_start(
                    out_ext[:, (2 * p + half) * QS:(2 * p + half + 1) * QS],
                    osb[:],
                )

            emit_s(0)
            emit_s(1)
            if pre is not None:
                pre()
            for j in range(nk):
                if filler is not None:
                    next(filler, None)
                if j + 2 < nk:
                    emit_s(j + 2)
                emit_pv(j)
                if j == nka - 1:
                    store_half(0)
            return lambda: store_half(1)

        # ---- schedule: one dense PE stream ----
        pqk0 = proj_t("pqk0")
        for _ in proj_half(0, wqk_sb, pqk0):
            pass
        drain_qk(0, pqk0)

        pv0 = proj_t("pv0")
        for _ in proj_half(0, wv_sb, pv0):
            pass

        pqk1 = proj_t("pqk1")
        store0b = attn(
            0,
            pre=lambda: vp_finish(0, pv0),
            filler=proj_half(1, wqk_sb, pqk1),
        )
        drain_qk(1, pqk1)
        store0b()

        pv1 = proj_t("pv1")
        for _ in proj_half(1, wv_sb, pv1):
            pass

        store1b = attn(1, pre=lambda: vp_finish(1, pv1))
        store1b()

    nc.compile()
    return nc


def make_inputs(x_b, Wq, bq, Wk, bk, Wv, bv):
    """Host-side prep for one core's in_map (x_b: [T, D] fp32)."""
    import ml_dtypes

    bf = ml_dtypes.bfloat16
    scale = 1.0 / np.sqrt(np.float32(HS))
    wqk = np.zeros((D, TT), dtype=np.float32)
    wqk[:, 0:HS] = Wq * scale
    wqk[:, HS:2 * HS] = Wk
    wv = np.zeros((D, TT), dtype=np.float32)
    wv[:, 0:HS] = Wv

    def chunk_major(w):
        # w2[p, c*128 + n] = w[c*128 + p, n] -> contiguous [128, 1024] DMA
        return np.ascontiguousarray(
            w.reshape(NDT, TT, TT).transpose(1, 0, 2).reshape(TT, NDT * TT)
        )

    bcol = np.zeros((TT, 2), dtype=np.float32)
    bcol[0:HS, 0] = bq * scale
    bcol[HS:2 * HS, 0] = bk
    bcol[0:HS, 1] = bv
    bcol[HS, 1] = 1.0
    identb = np.eye(TT, dtype=bf)
    return {
        "xt": np.ascontiguousarray(x_b.T).astype(bf),
        "wqk": chunk_major(wqk).astype(bf),
        "wv": chunk_major(wv).astype(bf),
        "bcol": bcol,
        "identb": identb,
    }


def finish_output(outT):
    """Host-side normalize + transpose: outT [65, T] -> [T, HS]."""
    o = np.asarray(outT, dtype=np.float32)
    return (o[0:HS, :] / o[HS:HS + 1, :]).T


_NC_CACHE = None


def _get_nc():
    global _NC_CACHE
    if _NC_CACHE is None:
        _NC_CACHE = build_graph()
    return _NC_CACHE


def kernel(x, Wq, bq, Wk, bk, Wv, bv):
    x = np.asarray(x, dtype=np.float32)
    args = [np.asarray(a, dtype=np.float32) for a in (Wq, bq, Wk, bk, Wv, bv)]
    nc = _get_nc()
    in_maps = [make_inputs(x[b], *args) for b in range(N_CORES)]
    trace = os.environ.get("BASS_ATTN_TRACE", "0") == "1"
    res = run_bass_kernel_spmd(
        nc, in_maps, core_ids=list(range(N_CORES)), trace=trace
    )
    if trace:
        print(
            f"HW exec time: {res.exec_time_ns} ns "
            f"(mean {res.mean_exec_time_ns}, max core {res.max_exec_time_core_id})"
        )
    out = np.stack(
        [finish_output(res.results[b]["outT"]) for b in range(N_CORES)], axis=0
    )
    return out
